# revision 35
# baseline (speedup 1.0000x reference)
"""Llama decoder layer (prefill, GQA, SwiGLU) on 8 Trainium2 NeuronCores.

Tensor-parallel across 8 cores, per the source model's sharding:
  - wq/wk/wv, w_gate/w_up column-sharded (4 q heads / 1 kv head / 1792 ffn per core)
  - wo, w_down row-sharded; AllReduce after o_proj, ReduceScatter after down_proj
  - ln1/ln2 weights folded into the following projection weights on host
  - matmuls bf16 (fp32 PSUM accum); norm stats/softmax in fp32; all HBM
    activation traffic and collectives in bf16

Schedule (v2):
  - attention computed transposed (sT[k,q] = K_kb^T-stationary x qT-moving)
    so probabilities come out already k-major for the PV matmul -> no
    per-k-block PE transposes; a ones-column appended to V yields the
    softmax denominator from the PV matmul itself.
  - RMSNorm1 applied via its row statistic only: x^T is pre-transposed on
    host, QKV runs on raw x^T, and rsqrt(mean(x^2)) is folded into the
    RoPE tables / V copy (RoPE is linear).
  - residual + RMSNorm2 fused into the per-512-row block pipeline right
    after each AllReduce chunk; h2^T bounced through DRAM in bf16.
  - residual output reconstructed on host as hidden + AllReduce output
    (the AR result is already in DRAM; no extra device traffic).

kernel(**inputs) takes the full unsharded fp32 inputs and returns
(h, residual) exactly like the reference decoder layer.
"""

import numpy as np
import ml_dtypes

import concourse.bass as bass
import concourse.mybir as mybir
import concourse.tile as tile
from concourse import bacc
from concourse.bass import ts, ds
from concourse.bass_utils import run_bass_kernel_spmd
from concourse.masks import make_identity

F32 = mybir.dt.float32
BF16 = mybir.dt.bfloat16
FP8 = mybir.dt.float8e4
DR = mybir.MatmulPerfMode.DoubleRow
WSC = 256.0          # fp8 weight pre-scale (2^8)
AF = mybir.ActivationFunctionType
ALU = mybir.AluOpType

HID = 4096
NH = 32
NKV = 8
HD = 128
G = 4            # q heads per kv head (= per core)
INTER = 14336
EPS = 1e-5
THETA = 10000.0
N_CORES = 8

FF = INTER // N_CORES       # 1792
FB = FF // 128              # 14 ffn 128-blocks
HC = HID // 128             # 32 hidden 128-chunks
EB = HID // 512             # 8 output 512-blocks
SCALING = float(HD) ** -0.5
BLK = 512                   # rows per pipeline block (= AR/RS chunk)


def _rms_finish(nc, pool_small, zc, rs_t):
    """zc [128,8] chunk sums-of-squares -> rs_t [128,1] rsqrt(mean+eps)."""
    ssq = pool_small.tile([128, 1], F32, tag="ssq")
    nc.vector.reduce_sum(ssq[:], zc[:], axis=mybir.AxisListType.X)
    nc.vector.tensor_scalar(rs_t[:], ssq[:], 1.0 / HID, EPS, ALU.mult, ALU.add)
    nc.scalar.sqrt(rs_t[:], rs_t[:])
    nc.vector.reciprocal(rs_t[:], rs_t[:])


def _build_program(S: int, no_collectives: bool = False):
    """Build the per-core Bass program (SPMD, rank-agnostic)."""
    T = S // 128
    NB = S // BLK                       # pipeline blocks
    TPB = BLK // 128                    # s-tiles per block (4)
    CHS = BLK // N_CORES                # scatter rows per core per block (64)
    HH = HID // 2                       # d-half width (2048)
    assert S % BLK == 0

    nc = bacc.Bacc("TRN2", target_bir_lowering=False, debug=False,
                   num_devices=N_CORES)

    # ---- I/O ----
    x_d = nc.dram_tensor("x_bf", [S, HID], BF16, kind="ExternalInput")
    xT_hi_d = nc.dram_tensor("xT_hi", [128, T, HC, 128], FP8,
                             kind="ExternalInput")
    xT_lo_d = nc.dram_tensor("xT_lo", [128, T, HC, 128], FP8,
                             kind="ExternalInput")
    cos_d = nc.dram_tensor("cos_t", [S, 64], F32, kind="ExternalInput")
    sin_d = nc.dram_tensor("sin_t", [S, 64], F32, kind="ExternalInput")
    mask_d = nc.dram_tensor("mask_diag", [128, TPB, BLK], BF16,
                            kind="ExternalInput")
    wqkv_hi_d = nc.dram_tensor("wqkv_hi", [128, HC // 2, 2, 768], FP8,
                               kind="ExternalInput")
    wqkv_lo_d = nc.dram_tensor("wqkv_lo", [128, HC // 2, 2, 768], FP8,
                               kind="ExternalInput")
    wo_d = nc.dram_tensor("wo_t", [128, G, HID], BF16, kind="ExternalInput")
    wg_hi_d = nc.dram_tensor("wg_hi", [FB, 128, HC // 2, 2, 128], FP8,
                             kind="ExternalInput")
    wg_lo_d = nc.dram_tensor("wg_lo", [FB, 128, HC // 2, 2, 128], FP8,
                             kind="ExternalInput")
    wu_hi_d = nc.dram_tensor("wu_hi", [FB, 128, HC // 2, 2, 128], FP8,
                             kind="ExternalInput")
    wu_lo_d = nc.dram_tensor("wu_lo", [FB, 128, HC // 2, 2, 128], FP8,
                             kind="ExternalInput")
    wd_hi_d = nc.dram_tensor("wd_hi", [EB, 128, FB // 2, 2, 512], FP8,
                             kind="ExternalInput")
    wd_lo_d = nc.dram_tensor("wd_lo", [EB, 128, FB // 2, 2, 512], FP8,
                             kind="ExternalInput")

    # ---- internal / output DRAM (per-block, so collectives pipeline) ----
    ar_in = [nc.dram_tensor(f"ar_in{j}", [BLK, HID], BF16) for j in range(NB)]
    ar_out = [nc.dram_tensor(f"ar_out{j}", [BLK, HID], BF16,
                             addr_space="Shared")
              for j in range(NB)]
    # the AR result doubles as the attention-branch output (host adds the
    # residual to reconstruct out_res)
    attn_out = [nc.dram_tensor(f"out_attn{j}", [BLK, HID], BF16,
                               kind="ExternalOutput") for j in range(NB)]
    rs_in = [nc.dram_tensor(f"rs_in{j}", [BLK, HID], BF16) for j in range(NB)]
    rs_out = [nc.dram_tensor(f"rs_out{j}", [CHS, HID], BF16)
              for j in range(NB)]
    h_out = [nc.dram_tensor(f"out_h{j}", [CHS, HID], BF16,
                            kind="ExternalOutput") for j in range(NB)]
    h2hi_d = [nc.dram_tensor(f"h2hi{j}", [128, 2, HC, 256], FP8)
              for j in range(NB)]
    h2lo_d = [nc.dram_tensor(f"h2lo{j}", [128, 2, HC, 256], FP8)
              for j in range(NB)]
    rg = [list(range(N_CORES))]

    with tile.TileContext(nc) as tc:
        with tc.tile_pool(name="persist", bufs=1) as persist, \
             tc.tile_pool(name="dstage", bufs=3) as dpool, \
             tc.tile_pool(name="h2stage", bufs=1) as h2pool, \
             tc.tile_pool(name="dsmall", bufs=3) as dsmall:
            ident = persist.tile([128, 128], BF16)
            make_identity(nc, ident[:])
            mask_sb = persist.tile([128, TPB, BLK], BF16)
            qT_sb = persist.tile([128, G, S], BF16)
            kT_sb = persist.tile([128, S], BF16)
            v_sb = persist.tile([128, T, 132], BF16)
            nc.vector.memset(v_sb[:, :, 128:132], 1.0)

            # ==== Phase A: RMSNorm1 stats + QKV + RoPE, per s-tile ====
            with (
                tc.tile_pool(name="stA", bufs=2) as stA,
                tc.tile_pool(name="stAs", bufs=3) as stAs,
                tc.tile_pool(name="stAw", bufs=1) as stAw,
                tc.tile_pool(name="stAp", bufs=3, space="PSUM") as psA,
                tc.tile_pool(name="stAq", bufs=3, space="PSUM") as psAq,
            ):
                def load_tile(i):
                    xt = stA.tile([128, HID], BF16, tag="xt", name="xt")
                    nc.scalar.dma_start(xt[:], x_d[ts(i, 128), :])
                    xTh = stA.tile([128, HC, 128], FP8, tag="xTh",
                                   name="xTh")
                    xTl = stA.tile([128, HC, 128], FP8, tag="xTl",
                                   name="xTl")
                    nc.scalar.dma_start(xTh[:], xT_hi_d[:, i])
                    nc.scalar.dma_start(xTl[:], xT_lo_d[:, i])
                    xTt = (xTh, xTl)
                    cs = stAs.tile([128, 64], F32, tag="cs", name="cs")
                    sn = stAs.tile([128, 64], F32, tag="sn", name="sn")
                    nc.scalar.dma_start(cs[:], cos_d[ts(i, 128), :])
                    nc.scalar.dma_start(sn[:], sin_d[ts(i, 128), :])
                    return xt, xTt, cs, sn

                pre = load_tile(0)
                wqh = stAw.tile([128, HC // 2, 2, 768], FP8, tag="wqh")
                wql = stAw.tile([128, HC // 2, 2, 768], FP8, tag="wql")
                for c8 in range(8):
                    nc.scalar.dma_start(wqh[:, ts(c8, 2)],
                                        wqkv_hi_d[:, ts(c8, 2)])
                    nc.scalar.dma_start(wql[:, ts(c8, 2)],
                                        wqkv_lo_d[:, ts(c8, 2)])
                for i in range(T):
                    xt, xTt, cs, sn = pre
                    if i + 1 < T:
                        pre = load_tile(i + 1)
                    zc = stAs.tile([128, EB], F32, tag="zc1")
                    sq = stAs.tile([128, 512], BF16, tag="sq1")
                    for c in range(EB):
                        nc.scalar.activation(sq[:], xt[:, ts(c, 512)],
                                             AF.Square,
                                             accum_out=zc[:, c:c + 1])
                    # rsqrt folded with the 2^-8 fp8-weight descale:
                    # rs = rsqrt((ssq/HID + EPS) * WSC^2) = rsqrt(m) / WSC
                    rs_t = stAs.tile([128, 1], F32, tag="rs1")
                    ssq = stAs.tile([128, 1], F32, tag="ssq1")
                    nc.vector.reduce_sum(ssq[:], zc[:],
                                         axis=mybir.AxisListType.X)
                    nc.vector.tensor_scalar(rs_t[:], ssq[:],
                                            WSC * WSC / HID,
                                            WSC * WSC * EPS,
                                            ALU.mult, ALU.add)
                    nc.scalar.sqrt(rs_t[:], rs_t[:])
                    nc.vector.reciprocal(rs_t[:], rs_t[:])

                    xTh, xTl = xTt
                    pq = psA.tile([128, 512], F32, tag="pq")
                    pkv = psAq.tile([128, 256], F32, tag="pkv")
                    for po, lo, hi2 in ((pq, 0, 512), (pkv, 512, 768)):
                        w = hi2 - lo
                        for cp in range(HC // 2):
                            xh = xTh[:, ds(2 * cp, 2), :]
                            xl = xTl[:, ds(2 * cp, 2), :]
                            wh = wqh[:, cp, :, lo:hi2]
                            wl = wql[:, cp, :, lo:hi2]
                            nc.tensor.matmul(po[:], xh, wh,
                                             start=(cp == 0), stop=False,
                                             perf_mode=DR)
                            nc.tensor.matmul(po[:], xl, wh,
                                             start=False, stop=False,
                                             perf_mode=DR)
                            nc.tensor.matmul(po[:], xh, wl,
                                             start=False,
                                             stop=(cp == HC // 2 - 1),
                                             perf_mode=DR)
                    # v = pv * rs (fold rmsnorm scale)
                    nc.vector.tensor_scalar_mul(v_sb[:, i, 0:128],
                                                pkv[:, 128:256], rs_t[:])

                    # fold rmsnorm scale into the rope tables for this tile
                    nc.vector.tensor_scalar_mul(cs[:], cs[:], rs_t[:])
                    nc.vector.tensor_scalar_mul(sn[:], sn[:], rs_t[:])

                    def rope(dst_bf, src_psum, nh):
                        s4 = src_psum.rearrange("p (h t d) -> p h t d",
                                                h=nh, t=2)
                        d4 = dst_bf.rearrange("p (h t d) -> p h t d",
                                              h=nh, t=2)
                        csb = cs[:, None, :].to_broadcast([128, nh, 64])
                        snb = sn[:, None, :].to_broadcast([128, nh, 64])
                        t1 = stAs.tile([128, nh, 64], F32, tag=f"rt1_{nh}")
                        t2 = stAs.tile([128, nh, 64], F32, tag=f"rt2_{nh}")
                        nc.vector.tensor_tensor(t1[:], s4[:, :, 0, :], csb,
                                                ALU.mult)
                        nc.vector.tensor_tensor(t2[:], s4[:, :, 1, :], snb,
                                                ALU.mult)
                        nc.vector.tensor_tensor(d4[:, :, 0, :], t1[:], t2[:],
                                                ALU.subtract)
                        nc.vector.tensor_tensor(t1[:], s4[:, :, 1, :], csb,
                                                ALU.mult)
                        nc.vector.tensor_tensor(t2[:], s4[:, :, 0, :], snb,
                                                ALU.mult)
                        nc.vector.tensor_tensor(d4[:, :, 1, :], t1[:], t2[:],
                                                ALU.add)

                    q_bf = stA.tile([128, 512], BF16, tag="qbf")
                    k_bf = stAs.tile([128, 128], BF16, tag="kbf")
                    rope(q_bf, pq, G)
                    rope(k_bf, pkv[:, 0:128], 1)
                    for h in range(G):
                        pt = psAq.tile([128, 128], BF16, tag="ptq",
                                       bufs=2)
                        nc.tensor.transpose(pt[:], q_bf[:, ts(h, 128)],
                                            ident[:])
                        nc.vector.tensor_copy(qT_sb[:, h, ts(i, 128)], pt[:])
                    pt = psAq.tile([128, 128], BF16, tag="ptq", bufs=2)
                    nc.tensor.transpose(pt[:], k_bf[:], ident[:])
                    nc.vector.tensor_copy(kT_sb[:, ts(i, 128)], pt[:])

            # ==== Phase B: attention + o_proj + AR + residual/RMSNorm2 ====
            with (
                tc.tile_pool(name="stB", bufs=3) as stB,
                tc.tile_pool(name="stBz", bufs=3) as stBz,
                tc.tile_pool(name="stBw", bufs=1) as stBw,
                tc.tile_pool(name="stBo", bufs=2) as stBo,
                tc.tile_pool(name="psS", bufs=3, space="PSUM") as psS,
                tc.tile_pool(name="psAtt", bufs=2, space="PSUM") as psAtt,
                tc.tile_pool(name="psO", bufs=2, space="PSUM") as psO,
                tc.tile_pool(name="psT", bufs=1, space="PSUM") as psT,
            ):
                wo_sb = stBw.tile([128, G, HID], BF16, tag="wo")
                for h in range(G):
                    nc.scalar.dma_start(wo_sb[:, h, :], wo_d[:, h, :])
                for j in range(TPB):
                    nc.scalar.dma_start(mask_sb[:, j, :], mask_d[:, j, :])

                def emit_D(b):
                    # residual + RMSNorm2 -> h2T DRAM (bf16), fused per tile
                    nc.sync.dma_start(attn_out[b][:], ar_out[b][:])
                    for pair in range(TPB // 2):
                        h2pair = h2pool.tile([128, HC, 256], BF16,
                                             tag="h2pair", name="h2pair")
                        for u in range(2):
                            ti = pair * 2 + u
                            i = b * TPB + ti
                            zc = dsmall.tile([128, EB], F32, tag="zc2",
                                             name="zc2")
                            sq = dsmall.tile([128, 512], BF16, tag="sq2",
                                             name="sq2")
                            res_h = []
                            for hf in range(2):
                                at = dpool.tile([128, HH], BF16, tag="at",
                                                name="at")
                                xt2 = dpool.tile([128, HH], BF16, tag="xt2",
                                                 name="xt2")
                                nc.sync.dma_start(
                                    at[:], ar_out[b][ts(ti, 128), ts(hf, HH)])
                                nc.scalar.dma_start(
                                    xt2[:], x_d[ts(i, 128), ts(hf, HH)])
                                nc.vector.tensor_tensor(xt2[:], at[:], xt2[:],
                                                        ALU.add)
                                for c in range(4):
                                    nc.scalar.activation(
                                        sq[:], xt2[:, ts(c, 512)], AF.Square,
                                        accum_out=zc[:, hf * 4 + c:
                                                     hf * 4 + c + 1])
                                res_h.append((at, xt2))
                            rs2 = dsmall.tile([128, 1], F32, tag="rs2",
                                              name="rs2")
                            _rms_finish(nc, dsmall, zc, rs2)
                            for hf in range(2):
                                at, xt2 = res_h[hf]
                                nc.vector.tensor_scalar_mul(at[:], xt2[:],
                                                            rs2[:])
                                nc.sync.dma_start_transpose(
                                    h2pair[:, ts(hf, HC // 2), ts(u, 128)],
                                    at[:])
                        h2hi_t = h2pool.tile([128, HC, 256], FP8,
                                             tag="h2hi", name="h2hi_t")
                        h2lo_t = h2pool.tile([128, HC, 256], FP8,
                                             tag="h2lo", name="h2lo_t")
                        nc.vector.tensor_copy(h2hi_t[:], h2pair[:])
                        nc.vector.tensor_tensor(h2lo_t[:], h2pair[:],
                                                h2hi_t[:], ALU.subtract)
                        nc.sync.dma_start(h2hi_d[b][:, pair], h2hi_t[:])
                        nc.sync.dma_start(h2lo_d[b][:, pair], h2lo_t[:])

                for b in range(NB):
                    nk = TPB * b + TPB          # k-tiles for this block
                    attnT = stBo.tile([128, G, BLK], BF16, tag="attnT")
                    for h in range(G):
                        pT = stB.tile([128, T, BLK], BF16, tag="pT",
                                      bufs=1)

                        def sT_step(kb):
                            sp = psS.tile([128, BLK], F32, tag="sT")
                            nc.tensor.matmul(sp[:], kT_sb[:, ts(kb, 128)],
                                             qT_sb[:, h, ts(b, BLK)],
                                             start=True, stop=True)
                            if kb >= TPB * b:
                                nc.vector.tensor_tensor(
                                    sp[:], sp[:], mask_sb[:, kb - TPB * b, :],
                                    ALU.add)
                            return sp

                        sps = [sT_step(0), sT_step(1) if nk > 1 else None]
                        for kb in range(nk):
                            if kb + 2 < nk:
                                sps.append(sT_step(kb + 2))
                            nc.scalar.activation(pT[:, kb, :], sps[kb][:],
                                                 AF.Exp, scale=SCALING)
                        for t in range(TPB):
                            att = psAtt.tile([128, 132], F32, tag="att")
                            for kb in range(nk):
                                nc.tensor.matmul(att[:, 0:129],
                                                 pT[:, kb, ts(t, 128)],
                                                 v_sb[:, kb, 0:129],
                                                 start=(kb == 0),
                                                 stop=(kb == nk - 1))
                            z = stBz.tile([128, 1], F32, tag="z")
                            nc.vector.reciprocal(z[:], att[:, 128:129])
                            a_bf = stBz.tile([128, 128], BF16, tag="abf")
                            nc.vector.tensor_scalar_mul(a_bf[:],
                                                        att[:, 0:128],
                                                        z[:])
                            pt2 = psT.tile([128, 128], BF16, tag="pta")
                            nc.tensor.transpose(pt2[:], a_bf[:], ident[:])
                            nc.vector.tensor_copy(attnT[:, h, ts(t, 128)],
                                                  pt2[:])

                    # o_proj for this block's tiles
                    for ti in range(TPB):
                        ot = stB.tile([128, HID], BF16, tag="ot", bufs=2)
                        for e in range(EB):
                            po = psO.tile([128, 512], F32, tag="po")
                            for h in range(G):
                                nc.tensor.matmul(po[:],
                                                 attnT[:, h, ts(ti, 128)],
                                                 wo_sb[:, h, ts(e, 512)],
                                                 start=(h == 0),
                                                 stop=(h == G - 1))
                            nc.vector.tensor_copy(ot[:, ts(e, 512)], po[:])
                        nc.sync.dma_start(ar_in[b][ts(ti, 128), :], ot[:])

                    if no_collectives:
                        nc.sync.dma_start(ar_out[b][:], ar_in[b][:])
                    else:
                        nc.gpsimd.collective_compute(
                            "AllReduce", ALU.add, ins=[ar_in[b][:]],
                            outs=[ar_out[b][:]], replica_groups=rg)
                    # D sections (residual + RMSNorm2) are emitted with a
                    # one-block delay so their AR-dependent reads never park
                    # long at a queue head while the AR is still in flight.
                    if b >= 1:
                        emit_D(b - 1)

            # ==== Phase C: MLP per block + RS ====
            with (
                tc.tile_pool(name="stC", bufs=2) as stC,
                tc.tile_pool(name="stCw", bufs=2) as stCw,
                tc.tile_pool(name="stCg", bufs=1) as stCg,
                tc.tile_pool(name="stCh", bufs=1) as stCh,
                tc.tile_pool(name="psG", bufs=2, space="PSUM") as psG,
                tc.tile_pool(name="psD", bufs=2, space="PSUM") as psD,
            ):
                def load_h2(b):
                    h2s = stCh.tile([128, 2, HC, 256], FP8, tag="h2s",
                                    name="h2s")
                    h2l = stCh.tile([128, 2, HC, 256], FP8, tag="h2l",
                                    name="h2l")
                    nc.scalar.dma_start(h2s[:], h2hi_d[b][:])
                    nc.scalar.dma_start(h2l[:], h2lo_d[b][:])
                    return h2s, h2l

                h2pairs = load_h2(0)
                for b in range(NB):
                    h2s, h2l = h2pairs
                    guT = stCg.tile([128, FB, BLK], FP8, tag="guT")
                    guL = stCg.tile([128, FB, BLK], FP8, tag="guL")
                    for f in range(FB):
                        wgh = stCw.tile([128, HC // 2, 2, 128], FP8, tag="wgh")
                        wgl = stCw.tile([128, HC // 2, 2, 128], FP8, tag="wgl", bufs=1)
                        wuh = stCw.tile([128, HC // 2, 2, 128], FP8, tag="wuh")
                        wul = stCw.tile([128, HC // 2, 2, 128], FP8, tag="wul", bufs=1)
                        nc.scalar.dma_start(wgh[:], wg_hi_d[f])
                        nc.scalar.dma_start(wgl[:], wg_lo_d[f])
                        nc.scalar.dma_start(wuh[:], wu_hi_d[f])
                        nc.scalar.dma_start(wul[:], wu_lo_d[f])
                        pg = psG.tile([128, 512], F32, tag="pg")
                        pu = psG.tile([128, 512], F32, tag="pu")
                        for ps_t, wh, wl in ((pg, wgh, wgl), (pu, wuh, wul)):
                            for p in range(2):
                                for cp in range(HC // 2):
                                    xh = h2s[:, p, ds(2 * cp, 2), :]
                                    xl = h2l[:, p, ds(2 * cp, 2), :]
                                    nc.tensor.matmul(ps_t[:, ts(p, 256)],
                                                     wh[:, cp], xh,
                                                     start=(cp == 0),
                                                     stop=False, perf_mode=DR)
                                    nc.tensor.matmul(ps_t[:, ts(p, 256)],
                                                     wl[:, cp], xh,
                                                     start=False, stop=False,
                                                     perf_mode=DR)
                                    nc.tensor.matmul(ps_t[:, ts(p, 256)],
                                                     wh[:, cp], xl,
                                                     start=False,
                                                     stop=(cp == HC // 2 - 1),
                                                     perf_mode=DR)
                        sil = stC.tile([128, 512], F32, tag="sil")
                        nc.scalar.activation(sil[:], pg[:], AF.Silu,
                                             scale=1.0 / WSC)
                        pu2 = stC.tile([128, 512], F32, tag="pu2")
                        nc.scalar.mul(pu2[:], pu[:], 1.0 / WSC)
                        gu_bf = stC.tile([128, 512], BF16, tag="gubf")
                        nc.vector.tensor_tensor(gu_bf[:], sil[:], pu2[:],
                                                ALU.mult)
                        nc.scalar.copy(guT[:, f, :], gu_bf[:])
                        nc.vector.tensor_tensor(guL[:, f, :], gu_bf[:],
                                                guT[:, f, :], ALU.subtract)
                    if b == 0:
                        emit_D(3)
                    # prefetch next block's h2 while the down-proj runs
                    if b + 1 < NB:
                        h2pairs_next = load_h2(b + 1)
                    for e in range(EB):
                        wdh = stCw.tile([128, FB // 2, 2, 512], FP8, tag="wdh")
                        wdl = stCw.tile([128, FB // 2, 2, 512], FP8, tag="wdl")
                        nc.scalar.dma_start(wdh[:], wd_hi_d[e])
                        nc.scalar.dma_start(wdl[:], wd_lo_d[e])
                        for ti in range(TPB):
                            pd = psD.tile([128, 512], F32, tag="pd")
                            for fp in range(FB // 2):
                                gh = guT[:, ds(2 * fp, 2), ts(ti, 128)]
                                gl = guL[:, ds(2 * fp, 2), ts(ti, 128)]
                                wh = wdh[:, fp]
                                wl = wdl[:, fp]
                                nc.tensor.matmul(pd[:], gh, wh,
                                                 start=(fp == 0),
                                                 stop=False, perf_mode=DR)
                                nc.tensor.matmul(pd[:], gl, wh,
                                                 start=False, stop=False,
                                                 perf_mode=DR)
                                nc.tensor.matmul(pd[:], gh, wl,
                                                 start=False,
                                                 stop=(fp == FB // 2 - 1),
                                                 perf_mode=DR)
                            od = stC.tile([128, 512], BF16, tag="od")
                            nc.scalar.mul(od[:], pd[:], 1.0 / WSC)
                            nc.sync.dma_start(
                                rs_in[b][ts(ti, 128), ts(e, 512)], od[:])
                    if no_collectives:
                        nc.sync.dma_start(rs_out[b][:], rs_in[b][0:CHS, :])
                    else:
                        nc.gpsimd.collective_compute(
                            "ReduceScatter", ALU.add, ins=[rs_in[b][:]],
                            outs=[rs_out[b][:]], replica_groups=rg)
                    if b > 0:
                        nc.sync.dma_start(h_out[b - 1][:], rs_out[b - 1][:])
                    if b == NB - 1:
                        nc.sync.dma_start(h_out[b][:], rs_out[b][:])
                    if b + 1 < NB:
                        h2pairs = h2pairs_next

    nc.compile()
    return nc


_PROGRAM_CACHE = {}


def _get_program(S):
    if S not in _PROGRAM_CACHE:
        _PROGRAM_CACHE[S] = _build_program(S)
    return _PROGRAM_CACHE[S]


def _prep_inputs(positions, hidden_states, wq, wk, wv, wo,
                 w_gate, w_up, w_down, ln1_w, ln2_w):
    """Shard + retile + cast weights per core. Returns list of in_maps."""
    bf = ml_dtypes.bfloat16
    S = np.asarray(hidden_states).shape[0]
    T = S // 128
    TPB = BLK // 128
    pos = np.asarray(positions, np.float32)
    half = HD // 2
    inv_freq = 1.0 / (THETA ** (np.arange(half, dtype=np.float32) * 2.0 / HD))
    freqs = pos[:, None] * inv_freq[None, :]
    cos_t = np.cos(freqs).astype(np.float32)
    sin_t = np.sin(freqs).astype(np.float32)

    # diagonal-block causal masks, transposed layout: mask[k, j, q'] for the
    # j-th diagonal k-tile of a 512-wide q block (q' spans 4 q-tiles)
    ki = np.arange(128)
    qi = np.arange(BLK)
    qt = qi // 128
    ql = qi % 128
    mask = np.empty((128, TPB, BLK), np.float32)
    for j in range(TPB):
        valid = (qt[None, :] > j) | ((qt[None, :] == j)
                                     & (ql[None, :] >= ki[:, None]))
        mask[:, j, :] = np.where(valid, 0.0, -1e9)
    mask = mask.astype(bf)

    ln1 = np.asarray(ln1_w, np.float32)[:, None]
    ln2 = np.asarray(ln2_w, np.float32)[:, None]
    wq_f = np.asarray(wq, np.float32) * ln1
    wk_f = np.asarray(wk, np.float32) * ln1
    wv_f = np.asarray(wv, np.float32) * ln1
    f8 = ml_dtypes.float8_e4m3fn

    def split8(w):
        ws = (w * 256.0).astype(np.float32)
        hi = ws.astype(f8)
        lo = (ws - hi.astype(np.float32)).astype(f8)
        return hi, lo

    wg_f = (np.asarray(w_gate, np.float32) * ln2)
    wu_f = (np.asarray(w_up, np.float32) * ln2)
    wo_f = np.asarray(wo).astype(bf)
    wd_f = np.asarray(w_down, np.float32)
    hid = np.asarray(hidden_states, np.float32)
    x_bf = np.ascontiguousarray(hid.astype(bf))
    f8_t = ml_dtypes.float8_e4m3fn
    # tile-major transposed x: xT[dl, i, c, sl] = x[i*128+sl, c*128+dl],
    # split into fp8 hi + lo (x is unit-scale; no pre-scaling needed)
    xT_f = np.ascontiguousarray(
        x_bf.astype(np.float32).reshape(T, 128, HC, 128).transpose(3, 0, 2, 1))
    xT_hi = xT_f.astype(f8_t)
    xT_lo = (xT_f - xT_hi.astype(np.float32)).astype(f8_t)

    maps = []
    for r in range(N_CORES):
        wq_r = wq_f[:, r * 512:(r + 1) * 512]
        wk_r = wk_f[:, r * 128:(r + 1) * 128]
        wv_r = wv_f[:, r * 128:(r + 1) * 128]
        wqkv = np.concatenate([wq_r, wk_r, wv_r], axis=1)        # [4096, 768]
        wqkv_hi, wqkv_lo = split8(wqkv)

        def qkv_tiles(w):
            t = w.reshape(HC, 128, 768).transpose(1, 0, 2)
            return np.ascontiguousarray(t.reshape(128, HC // 2, 2, 768))
        wo_r = wo_f[r * 512:(r + 1) * 512, :]                    # [512, 4096]
        wo_t = np.ascontiguousarray(
            wo_r.reshape(G, 128, HID).transpose(1, 0, 2))        # [128, 4, 4096]
        wg_r = wg_f[:, r * FF:(r + 1) * FF]                      # [4096, 1792]
        wu_r = wu_f[:, r * FF:(r + 1) * FF]

        def gu_tiles(w):
            # [FB, 128, HC, 128] -> DR pair layout [FB, 128, HC//2, 2, 128]
            t = w.reshape(HC, 128, FB, 128).transpose(2, 1, 0, 3)
            return np.ascontiguousarray(
                t.reshape(FB, 128, HC // 2, 2, 128))

        wg_hi, wg_lo = split8(wg_r)
        wu_hi, wu_lo = split8(wu_r)
        wd_r = wd_f[r * FF:(r + 1) * FF, :]                      # [1792, 4096]
        wd_hi, wd_lo = split8(wd_r)

        def wd_tiles(w):
            t = w.reshape(FB, 128, EB, 512).transpose(2, 1, 0, 3)
            return np.ascontiguousarray(
                t.reshape(EB, 128, FB // 2, 2, 512))

        maps.append({
            "x_bf": x_bf, "xT_hi": xT_hi, "xT_lo": xT_lo,
            "cos_t": cos_t, "sin_t": sin_t,
            "mask_diag": mask, "wqkv_hi": qkv_tiles(wqkv_hi),
            "wqkv_lo": qkv_tiles(wqkv_lo), "wo_t": wo_t,
            "wg_hi": gu_tiles(wg_hi), "wg_lo": gu_tiles(wg_lo),
            "wu_hi": gu_tiles(wu_hi), "wu_lo": gu_tiles(wu_lo),
            "wd_hi": wd_tiles(wd_hi), "wd_lo": wd_tiles(wd_lo),
        })
    return maps


def kernel(positions, hidden_states, wq, wk, wv, wo,
           w_gate, w_up, w_down, ln1_w, ln2_w):
    S = np.asarray(hidden_states).shape[0]
    nc = _get_program(S)
    maps = _prep_inputs(positions, hidden_states, wq, wk, wv, wo,
                        w_gate, w_up, w_down, ln1_w, ln2_w)
    res = run_bass_kernel_spmd(nc, maps, list(range(N_CORES)))
    NB = S // BLK
    CHS = BLK // N_CORES
    h = np.empty((S, HID), np.float32)
    for r in range(N_CORES):
        for j in range(NB):
            h[j * BLK + r * CHS:j * BLK + (r + 1) * CHS] = \
                np.asarray(res.results[r][f"out_h{j}"], np.float32)
    attn = np.concatenate(
        [np.asarray(res.results[0][f"out_attn{j}"], np.float32)
         for j in range(NB)], axis=0)
    residual = np.asarray(hidden_states, np.float32) + attn
    return h, residual


# revision 38
# speedup vs baseline: 1.0024x; 1.0024x over previous
"""Llama decoder layer (prefill, GQA, SwiGLU) on 8 Trainium2 NeuronCores.

Tensor-parallel across 8 cores, per the source model's sharding:
  - wq/wk/wv, w_gate/w_up column-sharded (4 q heads / 1 kv head / 1792 ffn per core)
  - wo, w_down row-sharded; AllReduce after o_proj, ReduceScatter after down_proj
  - ln1/ln2 weights folded into the following projection weights on host
  - matmuls bf16 (fp32 PSUM accum); norm stats/softmax in fp32; all HBM
    activation traffic and collectives in bf16

Schedule (v2):
  - attention computed transposed (sT[k,q] = K_kb^T-stationary x qT-moving)
    so probabilities come out already k-major for the PV matmul -> no
    per-k-block PE transposes; a ones-column appended to V yields the
    softmax denominator from the PV matmul itself.
  - RMSNorm1 applied via its row statistic only: x^T is pre-transposed on
    host, QKV runs on raw x^T, and rsqrt(mean(x^2)) is folded into the
    RoPE tables / V copy (RoPE is linear).
  - residual + RMSNorm2 fused into the per-512-row block pipeline right
    after each AllReduce chunk; h2^T bounced through DRAM in bf16.
  - residual output reconstructed on host as hidden + AllReduce output
    (the AR result is already in DRAM; no extra device traffic).

kernel(**inputs) takes the full unsharded fp32 inputs and returns
(h, residual) exactly like the reference decoder layer.
"""

import numpy as np
import ml_dtypes

import concourse.bass as bass
import concourse.mybir as mybir
import concourse.tile as tile
from concourse import bacc
from concourse.bass import ts, ds
from concourse.bass_utils import run_bass_kernel_spmd
from concourse.masks import make_identity

F32 = mybir.dt.float32
BF16 = mybir.dt.bfloat16
FP8 = mybir.dt.float8e4
DR = mybir.MatmulPerfMode.DoubleRow
WSC = 256.0          # fp8 weight pre-scale (2^8)
AF = mybir.ActivationFunctionType
ALU = mybir.AluOpType

HID = 4096
NH = 32
NKV = 8
HD = 128
G = 4            # q heads per kv head (= per core)
INTER = 14336
EPS = 1e-5
THETA = 10000.0
N_CORES = 8

FF = INTER // N_CORES       # 1792
FB = FF // 128              # 14 ffn 128-blocks
HC = HID // 128             # 32 hidden 128-chunks
EB = HID // 512             # 8 output 512-blocks
SCALING = float(HD) ** -0.5
BLK = 512                   # rows per pipeline block (= AR/RS chunk)


def _rms_finish(nc, pool_small, zc, rs_t):
    """zc [128,8] chunk sums-of-squares -> rs_t [128,1] rsqrt(mean+eps)."""
    ssq = pool_small.tile([128, 1], F32, tag="ssq")
    nc.vector.reduce_sum(ssq[:], zc[:], axis=mybir.AxisListType.X)
    nc.vector.tensor_scalar(rs_t[:], ssq[:], 1.0 / HID, EPS, ALU.mult, ALU.add)
    nc.scalar.sqrt(rs_t[:], rs_t[:])
    nc.vector.reciprocal(rs_t[:], rs_t[:])


def _build_program(S: int, no_collectives: bool = False):
    """Build the per-core Bass program (SPMD, rank-agnostic)."""
    T = S // 128
    NB = S // BLK                       # pipeline blocks
    TPB = BLK // 128                    # s-tiles per block (4)
    CHS = BLK // N_CORES                # scatter rows per core per block (64)
    HH = HID // 2                       # d-half width (2048)
    assert S % BLK == 0

    nc = bacc.Bacc("TRN2", target_bir_lowering=False, debug=False,
                   num_devices=N_CORES)

    # ---- I/O ----
    x_d = nc.dram_tensor("x_bf", [S, HID], BF16, kind="ExternalInput")
    xT_hi_d = nc.dram_tensor("xT_hi", [128, T, HC, 128], FP8,
                             kind="ExternalInput")
    xT_lo_d = nc.dram_tensor("xT_lo", [128, T, HC, 128], FP8,
                             kind="ExternalInput")
    cos_d = nc.dram_tensor("cos_t", [S, 64], F32, kind="ExternalInput")
    sin_d = nc.dram_tensor("sin_t", [S, 64], F32, kind="ExternalInput")
    mask_d = nc.dram_tensor("mask_diag", [128, TPB, BLK], BF16,
                            kind="ExternalInput")
    wqkv_hi_d = nc.dram_tensor("wqkv_hi", [128, HC // 2, 2, 768], FP8,
                               kind="ExternalInput")
    wqkv_lo_d = nc.dram_tensor("wqkv_lo", [128, HC // 2, 2, 768], FP8,
                               kind="ExternalInput")
    wo_d = nc.dram_tensor("wo_t", [128, G, HID], BF16, kind="ExternalInput")
    wg_hi_d = nc.dram_tensor("wg_hi", [FB, 128, HC // 2, 2, 128], FP8,
                             kind="ExternalInput")
    wg_lo_d = nc.dram_tensor("wg_lo", [FB, 128, HC // 2, 2, 128], FP8,
                             kind="ExternalInput")
    wu_hi_d = nc.dram_tensor("wu_hi", [FB, 128, HC // 2, 2, 128], FP8,
                             kind="ExternalInput")
    wu_lo_d = nc.dram_tensor("wu_lo", [FB, 128, HC // 2, 2, 128], FP8,
                             kind="ExternalInput")
    wd_hi_d = nc.dram_tensor("wd_hi", [EB, 128, FB // 2, 2, 512], FP8,
                             kind="ExternalInput")
    wd_lo_d = nc.dram_tensor("wd_lo", [EB, 128, FB // 2, 2, 512], FP8,
                             kind="ExternalInput")

    # ---- internal / output DRAM (per-block, so collectives pipeline) ----
    ar_in = [nc.dram_tensor(f"ar_in{j}", [BLK, HID], BF16) for j in range(NB)]
    ar_out = [nc.dram_tensor(f"ar_out{j}", [BLK, HID], BF16,
                             addr_space="Shared")
              for j in range(NB)]
    # the AR result doubles as the attention-branch output (host adds the
    # residual to reconstruct out_res)
    attn_out = [nc.dram_tensor(f"out_attn{j}", [BLK, HID], BF16,
                               kind="ExternalOutput") for j in range(NB)]
    rs_in = [nc.dram_tensor(f"rs_in{j}", [BLK, HID], BF16) for j in range(NB)]
    rs_out = [nc.dram_tensor(f"rs_out{j}", [CHS, HID], BF16)
              for j in range(NB)]
    h_out = [nc.dram_tensor(f"out_h{j}", [CHS, HID], BF16,
                            kind="ExternalOutput") for j in range(NB)]
    h2hi_d = [nc.dram_tensor(f"h2hi{j}", [128, 2, HC, 256], FP8)
              for j in range(NB)]
    h2lo_d = [nc.dram_tensor(f"h2lo{j}", [128, 2, HC, 256], FP8)
              for j in range(NB)]
    rg = [list(range(N_CORES))]

    with tile.TileContext(nc) as tc:
        with tc.tile_pool(name="persist", bufs=1) as persist, \
             tc.tile_pool(name="dstage", bufs=3) as dpool, \
             tc.tile_pool(name="h2stage", bufs=1) as h2pool, \
             tc.tile_pool(name="dsmall", bufs=3) as dsmall:
            ident = persist.tile([128, 128], BF16)
            make_identity(nc, ident[:])
            mask_sb = persist.tile([128, TPB, BLK], BF16)
            qT_sb = persist.tile([128, G, S], BF16)
            kT_sb = persist.tile([128, S], BF16)
            v_sb = persist.tile([128, T, 132], BF16)
            nc.vector.memset(v_sb[:, :, 128:132], 1.0)

            # ==== Phase A: RMSNorm1 stats + QKV + RoPE, per s-tile ====
            with (
                tc.tile_pool(name="stA", bufs=2) as stA,
                tc.tile_pool(name="stAs", bufs=3) as stAs,
                tc.tile_pool(name="stAw", bufs=1) as stAw,
                tc.tile_pool(name="stAp", bufs=3, space="PSUM") as psA,
                tc.tile_pool(name="stAq", bufs=3, space="PSUM") as psAq,
            ):
                def load_tile(i):
                    xt = stA.tile([128, HID], BF16, tag="xt", name="xt")
                    nc.scalar.dma_start(xt[:], x_d[ts(i, 128), :])
                    xTh = stA.tile([128, HC, 128], FP8, tag="xTh",
                                   name="xTh")
                    xTl = stA.tile([128, HC, 128], FP8, tag="xTl",
                                   name="xTl")
                    nc.scalar.dma_start(xTh[:], xT_hi_d[:, i])
                    nc.scalar.dma_start(xTl[:], xT_lo_d[:, i])
                    xTt = (xTh, xTl)
                    cs = stAs.tile([128, 64], F32, tag="cs", name="cs")
                    sn = stAs.tile([128, 64], F32, tag="sn", name="sn")
                    nc.scalar.dma_start(cs[:], cos_d[ts(i, 128), :])
                    nc.scalar.dma_start(sn[:], sin_d[ts(i, 128), :])
                    return xt, xTt, cs, sn

                pre = load_tile(0)
                wqh = stAw.tile([128, HC // 2, 2, 768], FP8, tag="wqh")
                wql = stAw.tile([128, HC // 2, 2, 768], FP8, tag="wql")
                for c8 in range(8):
                    nc.scalar.dma_start(wqh[:, ts(c8, 2)],
                                        wqkv_hi_d[:, ts(c8, 2)])
                    nc.scalar.dma_start(wql[:, ts(c8, 2)],
                                        wqkv_lo_d[:, ts(c8, 2)])
                for i in range(T):
                    xt, xTt, cs, sn = pre
                    if i + 1 < T:
                        pre = load_tile(i + 1)
                    zc = stAs.tile([128, EB], F32, tag="zc1")
                    sq = stAs.tile([128, 512], BF16, tag="sq1")
                    for c in range(EB):
                        nc.scalar.activation(sq[:], xt[:, ts(c, 512)],
                                             AF.Square,
                                             accum_out=zc[:, c:c + 1])
                    # rsqrt folded with the 2^-8 fp8-weight descale:
                    # rs = rsqrt((ssq/HID + EPS) * WSC^2) = rsqrt(m) / WSC
                    rs_t = stAs.tile([128, 1], F32, tag="rs1")
                    ssq = stAs.tile([128, 1], F32, tag="ssq1")
                    nc.vector.reduce_sum(ssq[:], zc[:],
                                         axis=mybir.AxisListType.X)
                    nc.vector.tensor_scalar(rs_t[:], ssq[:],
                                            WSC * WSC / HID,
                                            WSC * WSC * EPS,
                                            ALU.mult, ALU.add)
                    nc.scalar.sqrt(rs_t[:], rs_t[:])
                    nc.vector.reciprocal(rs_t[:], rs_t[:])

                    xTh, xTl = xTt
                    pq = psA.tile([128, 512], F32, tag="pq")
                    pkv = psAq.tile([128, 256], F32, tag="pkv")
                    for po, lo, hi2 in ((pq, 0, 512), (pkv, 512, 768)):
                        w = hi2 - lo
                        for cp in range(HC // 2):
                            xh = xTh[:, ds(2 * cp, 2), :]
                            xl = xTl[:, ds(2 * cp, 2), :]
                            wh = wqh[:, cp, :, lo:hi2]
                            wl = wql[:, cp, :, lo:hi2]
                            nc.tensor.matmul(po[:], xh, wh,
                                             start=(cp == 0), stop=False,
                                             perf_mode=DR)
                            nc.tensor.matmul(po[:], xl, wh,
                                             start=False, stop=False,
                                             perf_mode=DR)
                            nc.tensor.matmul(po[:], xh, wl,
                                             start=False,
                                             stop=(cp == HC // 2 - 1),
                                             perf_mode=DR)
                    # v = pv * rs (fold rmsnorm scale)
                    nc.vector.tensor_scalar_mul(v_sb[:, i, 0:128],
                                                pkv[:, 128:256], rs_t[:])

                    # fold rmsnorm scale into the rope tables for this tile
                    nc.vector.tensor_scalar_mul(cs[:], cs[:], rs_t[:])
                    nc.vector.tensor_scalar_mul(sn[:], sn[:], rs_t[:])

                    def rope(dst_bf, src_psum, nh):
                        s4 = src_psum.rearrange("p (h t d) -> p h t d",
                                                h=nh, t=2)
                        d4 = dst_bf.rearrange("p (h t d) -> p h t d",
                                              h=nh, t=2)
                        csb = cs[:, None, :].to_broadcast([128, nh, 64])
                        snb = sn[:, None, :].to_broadcast([128, nh, 64])
                        t1 = stAs.tile([128, nh, 64], F32, tag=f"rt1_{nh}")
                        t2 = stAs.tile([128, nh, 64], F32, tag=f"rt2_{nh}")
                        nc.vector.tensor_tensor(t1[:], s4[:, :, 0, :], csb,
                                                ALU.mult)
                        nc.vector.tensor_tensor(t2[:], s4[:, :, 1, :], snb,
                                                ALU.mult)
                        nc.vector.tensor_tensor(d4[:, :, 0, :], t1[:], t2[:],
                                                ALU.subtract)
                        nc.vector.tensor_tensor(t1[:], s4[:, :, 1, :], csb,
                                                ALU.mult)
                        nc.vector.tensor_tensor(t2[:], s4[:, :, 0, :], snb,
                                                ALU.mult)
                        nc.vector.tensor_tensor(d4[:, :, 1, :], t1[:], t2[:],
                                                ALU.add)

                    q_bf = stA.tile([128, 512], BF16, tag="qbf")
                    k_bf = stAs.tile([128, 128], BF16, tag="kbf")
                    rope(q_bf, pq, G)
                    rope(k_bf, pkv[:, 0:128], 1)
                    for h in range(G):
                        pt = psAq.tile([128, 128], BF16, tag="ptq",
                                       bufs=2)
                        nc.tensor.transpose(pt[:], q_bf[:, ts(h, 128)],
                                            ident[:])
                        nc.vector.tensor_copy(qT_sb[:, h, ts(i, 128)], pt[:])
                    pt = psAq.tile([128, 128], BF16, tag="ptq", bufs=2)
                    nc.tensor.transpose(pt[:], k_bf[:], ident[:])
                    nc.vector.tensor_copy(kT_sb[:, ts(i, 128)], pt[:])

            # ==== Phase B: attention + o_proj + AR + residual/RMSNorm2 ====
            with (
                tc.tile_pool(name="stB", bufs=3) as stB,
                tc.tile_pool(name="stBz", bufs=3) as stBz,
                tc.tile_pool(name="stBw", bufs=1) as stBw,
                tc.tile_pool(name="stBo", bufs=2) as stBo,
                tc.tile_pool(name="psS", bufs=3, space="PSUM") as psS,
                tc.tile_pool(name="psAtt", bufs=2, space="PSUM") as psAtt,
                tc.tile_pool(name="psO", bufs=2, space="PSUM") as psO,
                tc.tile_pool(name="psT", bufs=1, space="PSUM") as psT,
            ):
                wo_sb = stBw.tile([128, G, HID], BF16, tag="wo")
                for h in range(G):
                    nc.scalar.dma_start(wo_sb[:, h, :], wo_d[:, h, :])
                for j in range(TPB):
                    nc.scalar.dma_start(mask_sb[:, j, :], mask_d[:, j, :])

                def emit_D(b):
                    # residual + RMSNorm2 -> h2T DRAM (bf16), fused per tile
                    nc.sync.dma_start(attn_out[b][:], ar_out[b][:])
                    for pair in range(TPB // 2):
                        h2pair = h2pool.tile([128, HC, 256], BF16,
                                             tag="h2pair", name="h2pair")
                        for u in range(2):
                            ti = pair * 2 + u
                            i = b * TPB + ti
                            zc = dsmall.tile([128, EB], F32, tag="zc2",
                                             name="zc2")
                            sq = dsmall.tile([128, 512], BF16, tag="sq2",
                                             name="sq2")
                            res_h = []
                            for hf in range(2):
                                at = dpool.tile([128, HH], BF16, tag="at",
                                                name="at")
                                xt2 = dpool.tile([128, HH], BF16, tag="xt2",
                                                 name="xt2")
                                nc.sync.dma_start(
                                    at[:], ar_out[b][ts(ti, 128), ts(hf, HH)])
                                nc.scalar.dma_start(
                                    xt2[:], x_d[ts(i, 128), ts(hf, HH)])
                                nc.vector.tensor_tensor(xt2[:], at[:], xt2[:],
                                                        ALU.add)
                                for c in range(4):
                                    nc.scalar.activation(
                                        sq[:], xt2[:, ts(c, 512)], AF.Square,
                                        accum_out=zc[:, hf * 4 + c:
                                                     hf * 4 + c + 1])
                                res_h.append((at, xt2))
                            rs2 = dsmall.tile([128, 1], F32, tag="rs2",
                                              name="rs2")
                            _rms_finish(nc, dsmall, zc, rs2)
                            for hf in range(2):
                                at, xt2 = res_h[hf]
                                nc.vector.tensor_scalar_mul(at[:], xt2[:],
                                                            rs2[:])
                                nc.sync.dma_start_transpose(
                                    h2pair[:, ts(hf, HC // 2), ts(u, 128)],
                                    at[:])
                        h2hi_t = h2pool.tile([128, HC, 256], FP8,
                                             tag="h2hi", name="h2hi_t")
                        h2lo_t = h2pool.tile([128, HC, 256], FP8,
                                             tag="h2lo", name="h2lo_t")
                        nc.vector.tensor_copy(h2hi_t[:], h2pair[:])
                        nc.vector.tensor_tensor(h2lo_t[:], h2pair[:],
                                                h2hi_t[:], ALU.subtract)
                        nc.sync.dma_start(h2hi_d[b][:, pair], h2hi_t[:])
                        nc.sync.dma_start(h2lo_d[b][:, pair], h2lo_t[:])

                for b in range(NB):
                    nk = TPB * b + TPB          # k-tiles for this block
                    attnT = stBo.tile([128, G, BLK], BF16, tag="attnT")
                    for h in range(G):
                        pT = stB.tile([128, T, BLK], BF16, tag="pT",
                                      bufs=1)

                        def sT_step(kb):
                            sp = psS.tile([128, BLK], F32, tag="sT")
                            nc.tensor.matmul(sp[:], kT_sb[:, ts(kb, 128)],
                                             qT_sb[:, h, ts(b, BLK)],
                                             start=True, stop=True)
                            if kb >= TPB * b:
                                nc.vector.tensor_tensor(
                                    sp[:], sp[:], mask_sb[:, kb - TPB * b, :],
                                    ALU.add)
                            return sp

                        sps = [sT_step(0), sT_step(1) if nk > 1 else None]
                        for kb in range(nk):
                            if kb + 2 < nk:
                                sps.append(sT_step(kb + 2))
                            nc.scalar.activation(pT[:, kb, :], sps[kb][:],
                                                 AF.Exp, scale=SCALING)
                        for t in range(TPB):
                            att = psAtt.tile([128, 132], F32, tag="att")
                            for kb in range(nk):
                                nc.tensor.matmul(att[:, 0:129],
                                                 pT[:, kb, ts(t, 128)],
                                                 v_sb[:, kb, 0:129],
                                                 start=(kb == 0),
                                                 stop=(kb == nk - 1))
                            z = stBz.tile([128, 1], F32, tag="z")
                            nc.vector.reciprocal(z[:], att[:, 128:129])
                            a_bf = stBz.tile([128, 128], BF16, tag="abf")
                            nc.vector.tensor_scalar_mul(a_bf[:],
                                                        att[:, 0:128],
                                                        z[:])
                            pt2 = psT.tile([128, 128], BF16, tag="pta")
                            nc.tensor.transpose(pt2[:], a_bf[:], ident[:])
                            nc.vector.tensor_copy(attnT[:, h, ts(t, 128)],
                                                  pt2[:])

                    # o_proj for this block's tiles
                    for ti in range(TPB):
                        ot = stB.tile([128, HID], BF16, tag="ot", bufs=2)
                        for e in range(EB):
                            po = psO.tile([128, 512], F32, tag="po")
                            for h in range(G):
                                nc.tensor.matmul(po[:],
                                                 attnT[:, h, ts(ti, 128)],
                                                 wo_sb[:, h, ts(e, 512)],
                                                 start=(h == 0),
                                                 stop=(h == G - 1))
                            nc.vector.tensor_copy(ot[:, ts(e, 512)], po[:])
                        nc.sync.dma_start(ar_in[b][ts(ti, 128), :], ot[:])

                    if no_collectives:
                        nc.sync.dma_start(ar_out[b][:], ar_in[b][:])
                    else:
                        nc.gpsimd.collective_compute(
                            "AllReduce", ALU.add, ins=[ar_in[b][:]],
                            outs=[ar_out[b][:]], replica_groups=rg)
                    # D sections (residual + RMSNorm2) are emitted with a
                    # one-block delay so their AR-dependent reads never park
                    # long at a queue head while the AR is still in flight.
                    if b >= 1:
                        emit_D(b - 1)

            # ==== Phase C: MLP per block + RS ====
            with (
                tc.tile_pool(name="stC", bufs=2) as stC,
                tc.tile_pool(name="stCw", bufs=2) as stCw,
                tc.tile_pool(name="stCg", bufs=1) as stCg,
                tc.tile_pool(name="stCh", bufs=1) as stCh,
                tc.tile_pool(name="psG", bufs=2, space="PSUM") as psG,
                tc.tile_pool(name="psD", bufs=2, space="PSUM") as psD,
            ):
                def load_h2(b):
                    h2s = stCh.tile([128, 2, HC, 256], FP8, tag="h2s",
                                    name="h2s")
                    h2l = stCh.tile([128, 2, HC, 256], FP8, tag="h2l",
                                    name="h2l")
                    nc.scalar.dma_start(h2s[:], h2hi_d[b][:])
                    nc.scalar.dma_start(h2l[:], h2lo_d[b][:])
                    return h2s, h2l

                h2pairs = load_h2(0)
                for b in range(NB):
                    h2s, h2l = h2pairs
                    guT = stCg.tile([128, FB, BLK], FP8, tag="guT")
                    guL = stCg.tile([128, FB, BLK], FP8, tag="guL")
                    for f in range(FB):
                        wgh = stCw.tile([128, HC // 2, 2, 128], FP8, tag="wgh")
                        wgl = stCw.tile([128, HC // 2, 2, 128], FP8, tag="wgl", bufs=1)
                        wuh = stCw.tile([128, HC // 2, 2, 128], FP8, tag="wuh")
                        wul = stCw.tile([128, HC // 2, 2, 128], FP8, tag="wul", bufs=1)
                        nc.scalar.dma_start(wgh[:], wg_hi_d[f])
                        nc.scalar.dma_start(wgl[:], wg_lo_d[f])
                        nc.scalar.dma_start(wuh[:], wu_hi_d[f])
                        nc.scalar.dma_start(wul[:], wu_lo_d[f])
                        pg = psG.tile([128, 512], F32, tag="pg")
                        pu = psG.tile([128, 512], F32, tag="pu")
                        for ps_t, wh, wl in ((pg, wgh, wgl), (pu, wuh, wul)):
                            for p in range(2):
                                for cp in range(HC // 2):
                                    xh = h2s[:, p, ds(2 * cp, 2), :]
                                    xl = h2l[:, p, ds(2 * cp, 2), :]
                                    nc.tensor.matmul(ps_t[:, ts(p, 256)],
                                                     wh[:, cp], xh,
                                                     start=(cp == 0),
                                                     stop=False, perf_mode=DR)
                                    nc.tensor.matmul(ps_t[:, ts(p, 256)],
                                                     wl[:, cp], xh,
                                                     start=False, stop=False,
                                                     perf_mode=DR)
                                    nc.tensor.matmul(ps_t[:, ts(p, 256)],
                                                     wh[:, cp], xl,
                                                     start=False,
                                                     stop=(cp == HC // 2 - 1),
                                                     perf_mode=DR)
                        sil = stC.tile([128, 512], F32, tag="sil")
                        nc.scalar.activation(sil[:], pg[:], AF.Silu,
                                             scale=1.0 / WSC)
                        pu2 = stC.tile([128, 512], F32, tag="pu2")
                        nc.scalar.mul(pu2[:], pu[:], 1.0 / WSC)
                        gu_bf = stC.tile([128, 512], BF16, tag="gubf")
                        nc.vector.tensor_tensor(gu_bf[:], sil[:], pu2[:],
                                                ALU.mult)
                        nc.scalar.copy(guT[:, f, :], gu_bf[:])
                        nc.vector.tensor_tensor(guL[:, f, :], gu_bf[:],
                                                guT[:, f, :], ALU.subtract)
                    if b == 0:
                        emit_D(3)
                    # prefetch next block's h2 while the down-proj runs
                    if b + 1 < NB:
                        h2pairs_next = load_h2(b + 1)
                    for e in range(EB):
                        wdh = stCw.tile([128, FB // 2, 2, 512], FP8, tag="wdh")
                        wdl = stCw.tile([128, FB // 2, 2, 512], FP8, tag="wdl")
                        nc.scalar.dma_start(wdh[:], wd_hi_d[e])
                        nc.scalar.dma_start(wdl[:], wd_lo_d[e])
                        for ti in range(TPB):
                            pd = psD.tile([128, 512], F32, tag="pd")
                            for fp in range(FB // 2):
                                gh = guT[:, ds(2 * fp, 2), ts(ti, 128)]
                                gl = guL[:, ds(2 * fp, 2), ts(ti, 128)]
                                wh = wdh[:, fp]
                                wl = wdl[:, fp]
                                nc.tensor.matmul(pd[:], gh, wh,
                                                 start=(fp == 0),
                                                 stop=False, perf_mode=DR)
                                nc.tensor.matmul(pd[:], gl, wh,
                                                 start=False, stop=False,
                                                 perf_mode=DR)
                                nc.tensor.matmul(pd[:], gh, wl,
                                                 start=False,
                                                 stop=(fp == FB // 2 - 1),
                                                 perf_mode=DR)
                            od = stC.tile([128, 512], BF16, tag="od")
                            nc.scalar.mul(od[:], pd[:], 1.0 / WSC)
                            nc.sync.dma_start(
                                rs_in[b][ts(ti, 128), ts(e, 512)], od[:])
                    if no_collectives:
                        nc.sync.dma_start(rs_out[b][:], rs_in[b][0:CHS, :])
                    else:
                        nc.gpsimd.collective_compute(
                            "ReduceScatter", ALU.add, ins=[rs_in[b][:]],
                            outs=[rs_out[b][:]], replica_groups=rg)
                    if b > 0:
                        nc.sync.dma_start(h_out[b - 1][:], rs_out[b - 1][:])
                    if b == NB - 1:
                        nc.sync.dma_start(h_out[b][:], rs_out[b][:])
                    if b + 1 < NB:
                        h2pairs = h2pairs_next

    nc.compile()
    return nc


_PROGRAM_CACHE = {}


def _get_program(S):
    if S not in _PROGRAM_CACHE:
        _PROGRAM_CACHE[S] = _build_program(S)
    return _PROGRAM_CACHE[S]


def _prep_inputs(positions, hidden_states, wq, wk, wv, wo,
                 w_gate, w_up, w_down, ln1_w, ln2_w):
    """Shard + retile + cast weights per core. Returns list of in_maps."""
    bf = ml_dtypes.bfloat16
    S = np.asarray(hidden_states).shape[0]
    T = S // 128
    TPB = BLK // 128
    pos = np.asarray(positions, np.float32)
    half = HD // 2
    inv_freq = 1.0 / (THETA ** (np.arange(half, dtype=np.float32) * 2.0 / HD))
    freqs = pos[:, None] * inv_freq[None, :]
    cos_t = np.cos(freqs).astype(np.float32)
    sin_t = np.sin(freqs).astype(np.float32)

    # diagonal-block causal masks, transposed layout: mask[k, j, q'] for the
    # j-th diagonal k-tile of a 512-wide q block (q' spans 4 q-tiles)
    ki = np.arange(128)
    qi = np.arange(BLK)
    qt = qi // 128
    ql = qi % 128
    mask = np.empty((128, TPB, BLK), np.float32)
    for j in range(TPB):
        valid = (qt[None, :] > j) | ((qt[None, :] == j)
                                     & (ql[None, :] >= ki[:, None]))
        mask[:, j, :] = np.where(valid, 0.0, -1e9)
    mask = mask.astype(bf)

    ln1 = np.asarray(ln1_w, np.float32)[:, None]
    ln2 = np.asarray(ln2_w, np.float32)[:, None]
    wq_f = np.asarray(wq, np.float32) * ln1
    wk_f = np.asarray(wk, np.float32) * ln1
    wv_f = np.asarray(wv, np.float32) * ln1
    f8 = ml_dtypes.float8_e4m3fn

    def split8(w):
        ws = (w * 256.0).astype(np.float32)
        hi = ws.astype(f8)
        lo = (ws - hi.astype(np.float32)).astype(f8)
        return hi, lo

    wg_f = (np.asarray(w_gate, np.float32) * ln2)
    wu_f = (np.asarray(w_up, np.float32) * ln2)
    wo_f = np.asarray(wo).astype(bf)
    wd_f = np.asarray(w_down, np.float32)
    hid = np.asarray(hidden_states, np.float32)
    x_bf = np.ascontiguousarray(hid.astype(bf))
    f8_t = ml_dtypes.float8_e4m3fn
    # tile-major transposed x: xT[dl, i, c, sl] = x[i*128+sl, c*128+dl],
    # split into fp8 hi + lo (x is unit-scale; no pre-scaling needed)
    xT_f = np.ascontiguousarray(
        x_bf.astype(np.float32).reshape(T, 128, HC, 128).transpose(3, 0, 2, 1))
    xT_hi = xT_f.astype(f8_t)
    xT_lo = (xT_f - xT_hi.astype(np.float32)).astype(f8_t)

    maps = []
    for r in range(N_CORES):
        wq_r = wq_f[:, r * 512:(r + 1) * 512]
        wk_r = wk_f[:, r * 128:(r + 1) * 128]
        wv_r = wv_f[:, r * 128:(r + 1) * 128]
        wqkv = np.concatenate([wq_r, wk_r, wv_r], axis=1)        # [4096, 768]
        wqkv_hi, wqkv_lo = split8(wqkv)

        def qkv_tiles(w):
            t = w.reshape(HC, 128, 768).transpose(1, 0, 2)
            return np.ascontiguousarray(t.reshape(128, HC // 2, 2, 768))
        wo_r = wo_f[r * 512:(r + 1) * 512, :]                    # [512, 4096]
        wo_t = np.ascontiguousarray(
            wo_r.reshape(G, 128, HID).transpose(1, 0, 2))        # [128, 4, 4096]
        wg_r = wg_f[:, r * FF:(r + 1) * FF]                      # [4096, 1792]
        wu_r = wu_f[:, r * FF:(r + 1) * FF]

        def gu_tiles(w):
            # [FB, 128, HC, 128] -> DR pair layout [FB, 128, HC//2, 2, 128]
            t = w.reshape(HC, 128, FB, 128).transpose(2, 1, 0, 3)
            return np.ascontiguousarray(
                t.reshape(FB, 128, HC // 2, 2, 128))

        wg_hi, wg_lo = split8(wg_r)
        wu_hi, wu_lo = split8(wu_r)
        wd_r = wd_f[r * FF:(r + 1) * FF, :]                      # [1792, 4096]
        wd_hi, wd_lo = split8(wd_r)

        def wd_tiles(w):
            t = w.reshape(FB, 128, EB, 512).transpose(2, 1, 0, 3)
            return np.ascontiguousarray(
                t.reshape(EB, 128, FB // 2, 2, 512))

        maps.append({
            "x_bf": x_bf, "xT_hi": xT_hi, "xT_lo": xT_lo,
            "cos_t": cos_t, "sin_t": sin_t,
            "mask_diag": mask, "wqkv_hi": qkv_tiles(wqkv_hi),
            "wqkv_lo": qkv_tiles(wqkv_lo), "wo_t": wo_t,
            "wg_hi": gu_tiles(wg_hi), "wg_lo": gu_tiles(wg_lo),
            "wu_hi": gu_tiles(wu_hi), "wu_lo": gu_tiles(wu_lo),
            "wd_hi": wd_tiles(wd_hi), "wd_lo": wd_tiles(wd_lo),
        })
    return maps


def kernel(positions, hidden_states, wq, wk, wv, wo,
           w_gate, w_up, w_down, ln1_w, ln2_w):
    S = np.asarray(hidden_states).shape[0]
    nc = _get_program(S)
    maps = _prep_inputs(positions, hidden_states, wq, wk, wv, wo,
                        w_gate, w_up, w_down, ln1_w, ln2_w)
    res = run_bass_kernel_spmd(nc, maps, list(range(N_CORES)))
    NB = S // BLK
    CHS = BLK // N_CORES
    h = np.empty((S, HID), np.float32)
    for r in range(N_CORES):
        for j in range(NB):
            h[j * BLK + r * CHS:j * BLK + (r + 1) * CHS] = \
                np.asarray(res.results[r][f"out_h{j}"], np.float32)
    attn = np.concatenate(
        [np.asarray(res.results[0][f"out_attn{j}"], np.float32)
         for j in range(NB)], axis=0)
    residual = np.asarray(hidden_states, np.float32) + attn
    return h, residual
